# revision 1
# baseline (speedup 1.0000x reference)
"""NaiveFourierKANLayer Trainium2 kernel (8-core SPMD, data-parallel over batch).

Math (per batch b):
  ff[n,o]  = sum_{d,c,g} trig_d(V[n,c]*k_g) * coeffs[d,o,c,g]   (k_g = g+1)
  out[i,o] = sum_{c,j} A[j,c,i] * ff[j,o] + bias[o]

Per core (2 batches/core), the pipeline is software-pipelined so the prep of
batch i+1 (V transpose -> range-reduced cos/sin features -> ff matmul) runs
on DVE/ACT/PE while the A-stream matmul of batch i (the 16 MB/batch roofline
term) is in flight.

  - cs features live as cs^T [2048, 1024]: 16 tiles of [128, 1024]; partition
    p of tile t is contraction row t*128+p, row order (d, g, c), c fastest.
    Range reduction (|kx| reaches ~80; the ACT Sin spline is only accurate on
    [-pi, pi]) is one fused custom DVE op per tile:
    d = t - round(t), t = V*(k/2pi) + phase, round() via the fp32
    magic-number trick; then ACT Sin(2pi*d).
  - ff = cs^T.T @ W on PE (16 K-chunks), psum [n=128, o=128].
  - main matmul folds the c-reduction into the contraction:
    out[i,o] = sum_{(c,jc)} A_chunk.T @ ff[jc]; A streams as 4 MB contiguous
    DMAs ([128 j, 2*4096 f32], two j-blocks per chunk).
  - bias is added via a rank-1 matmul (ones[1,128].T @ bias2[1,256]) opening
    each PSUM accumulation group.
"""

import numpy as np

import concourse.bacc as bacc
import concourse.tile as tile
from concourse import mybir
from concourse.bass import ts
from concourse.bass_utils import run_bass_kernel_spmd

B, N, C, IN, OUT, G = 16, 1024, 4, 64, 128, 16
N_CORES = 8
B_LOC = B // N_CORES
MAGIC = 12582912.0  # 1.5 * 2**23 : fp32 round-to-nearest-integer magic
TWO_PI = float(2.0 * np.pi)
F32 = mybir.dt.float32

NT = N // 128  # 8 n-tiles / j-chunks / i-tiles
KT = 2 * G * IN // 128  # 16 contraction chunks for the ff matmul


def _register_frac_op():
    """out = t - round(t), t = in0*s0 + s1; round via (t+MAGIC)-MAGIC."""
    import concourse.dve_ops as dvo
    from concourse.dve_spec import Spec, Src0, C0, C1, C2, lower
    from concourse.dve_uop import DveOpSpec

    name = "FRAC_KAN_ANT"
    for op in dvo.OPS:
        if op.name == name:
            return op

    def _ref(in0, in1, s0, s1, imm2):
        t = (np.float32(in0) * np.float32(s0) + np.float32(s1)).astype(np.float32)
        n = ((t + np.float32(imm2)).astype(np.float32) - np.float32(imm2)).astype(
            np.float32
        )
        return (t - n).astype(np.float32)

    t = Src0 * C0 + C1
    n = (t + C2) - C2
    spec = Spec(body=t - n, reference=_ref)
    placeholder = dvo.DveOp(name, spec, subdim=False, uops_sha={})
    dvo.OPS.append(placeholder)
    dvo._SUB_OPCODE_FOR_NAME[name] = dvo._CUSTOM_DVE_ROW_BASE + len(dvo.OPS) - 1
    dvo.CUSTOM_DVE_SPECS[name] = spec
    shas = {}
    for ver in ("v3", "v4"):
        try:
            ds = DveOpSpec(
                name=name,
                opcode=dvo.get_dve_sub_opcode(name),
                uops=lower(spec, ver=ver),
                rd1_en=False,
            )
            shas[ver] = ds.sha(ver)
        except Exception:
            pass
    final = dvo.DveOp(name, spec, subdim=False, uops_sha=shas)
    dvo.OPS[-1] = final
    return final


_NC_CACHE = {}


def build_nc(reps=1, mode="full", loop=False):
    key = (reps, mode, loop)
    if key in _NC_CACHE:
        return _NC_CACHE[key]
    frac_op = _register_frac_op()

    nc = bacc.Bacc("TRN2", target_bir_lowering=False, debug=False)
    Vd = nc.dram_tensor("V", [B_LOC, N, IN], F32, kind="ExternalInput")
    Ad = nc.dram_tensor("A", [B_LOC, N, C, N], F32, kind="ExternalInput")
    Wd = nc.dram_tensor("W", [2 * G * IN, OUT], F32, kind="ExternalInput")
    KSd = nc.dram_tensor("kscale", [128, G // 2], F32, kind="ExternalInput")
    IDd = nc.dram_tensor("ident", [128, 128], F32, kind="ExternalInput")
    B2d = nc.dram_tensor("bias2", [1, 2 * OUT], F32, kind="ExternalInput")
    Od = nc.dram_tensor("out", [B_LOC, N, OUT], F32, kind="ExternalOutput")

    if loop:
        items = list(range(B_LOC))  # one pass per For_i iteration
    else:
        items = [b for _ in range(reps) for b in range(B_LOC)]
    L = len(items)

    with tile.TileContext(nc) as tc:
        with (
            tc.tile_pool(name="const", bufs=1) as constp,
            tc.tile_pool(name="v", bufs=2) as vpool,
            tc.tile_pool(name="v2", bufs=2) as v2pool,
            tc.tile_pool(name="d", bufs=2) as dpool,
            tc.tile_pool(name="cs", bufs=1) as cspool,
            tc.tile_pool(name="ff", bufs=2) as ffpool,
            tc.tile_pool(name="a", bufs=16) as apool,
            tc.tile_pool(name="o", bufs=4) as opool,
            tc.tile_pool(name="ptr", bufs=2, space="PSUM") as ptrp,
            tc.tile_pool(name="pff", bufs=2, space="PSUM") as pffp,
            tc.tile_pool(name="pm", bufs=4, space="PSUM") as pmp,
        ):
            w_sb = constp.tile([128, KT * OUT], F32)
            nc.sync.dma_start(
                w_sb[:].rearrange("p (t o) -> p t o", t=KT),
                Wd.rearrange("(t p) o -> p t o", p=128),
            )
            ks_sb = constp.tile([128, G // 2], F32)
            nc.sync.dma_start(ks_sb[:], KSd[:])
            id_sb = constp.tile([128, 128], F32)
            nc.sync.dma_start(id_sb[:], IDd[:])
            b2_sb = constp.tile([1, 2 * OUT], F32)
            nc.sync.dma_start(b2_sb[:], B2d[:])
            ones_sb = constp.tile([1, OUT], F32)
            nc.vector.memset(ones_sb[:], 1.0)

            def a_csum_ap(b, jc, c):
                # [128 j, 1024 n] slice of A[b] at (j-block jc, c)
                return Ad[b].rearrange("(t p) c n -> t c p n", p=128)[jc][c]

            def emit_a_load(i):
                """A-sum tiles for item i: accumulate over c during the DMA.
                c=0 (plain write) rides HWDGE; c=1..3 accumulate via SWDGE."""
                b = items[i]
                a_tiles = [
                    apool.tile([128, N], F32, name=f"a_{i}_{jc}", tag="a")
                    for jc in range(NT)
                ]
                for jc in range(NT):
                    nc.sync.dma_start(a_tiles[jc][:], a_csum_ap(b, jc, 0))
                for c in range(1, C):
                    for jc in range(NT):
                        nc.gpsimd.dma_start(
                            a_tiles[jc][:],
                            a_csum_ap(b, jc, c),
                            accum_op=mybir.AluOpType.add,
                        )
                return a_tiles

            if mode == "dma":
                # A-stream-only floor: same DMA traffic + c-accum, no compute.
                acc = constp.tile([128, 16], F32)

                def emit_dma_floor():
                    for i, b in enumerate(items):
                        a_tiles = emit_a_load(i)
                        for jc in range(NT):
                            col = (i * NT + jc) % 16
                            nc.vector.reduce_sum(
                                acc[:, col : col + 1],
                                a_tiles[jc][:, 0:512],
                                axis=mybir.AxisListType.X,
                            )

                if loop:
                    with tc.For_i(0, reps, 1):
                        emit_dma_floor()
                else:
                    emit_dma_floor()
                nc_done = True
            else:
                nc_done = False

            def emit_prep(i):
                """V load + transpose + frac/sin features + cs for item i."""
                b = items[i]
                v_sb = vpool.tile([128, NT * IN], F32, name=f"v_{i}", tag="v")
                nc.sync.dma_start(
                    v_sb[:].rearrange("p (t c) -> p t c", t=NT),
                    Vd[b].rearrange("(t p) c -> p t c", p=128),
                )
                v2 = v2pool.tile([128, N], F32, name=f"v2_{i}", tag="v2")
                for t8 in range(NT):
                    ptr = ptrp.tile([IN, 128], F32, name=f"ptr_{i}_{t8}", tag="ptr")
                    nc.tensor.transpose(ptr[:], v_sb[:, ts(t8, IN)], id_sb[:])
                    nc.vector.tensor_copy(v2[0:IN, ts(t8, 128)], ptr[:])
                    nc.vector.tensor_copy(v2[IN : 2 * IN, ts(t8, 128)], ptr[:])
                return v2

            CSG = 4  # K-chunks per cs tile: Tile deps are tile-granular, so
            # smaller cs tiles let ff matmuls start before ALL sins finish.

            def emit_cs(i, v2):
                cs_groups = [
                    cspool.tile(
                        [128, CSG * N], F32, name=f"cs_{i}_{g}", tag=f"cs{g}"
                    )
                    for g in range(KT // CSG)
                ]
                for t16 in range(KT):
                    gp = t16 % NT
                    phase = 0.25 if t16 < 8 else 0.0  # tiles 0..7 = cos
                    d = dpool.tile([128, N], F32, name=f"d_{i}_{t16}", tag="d")
                    nc.vector._custom_dve(
                        frac_op,
                        out=d[:],
                        in0=v2[:],
                        s0=ks_sb[:, gp : gp + 1],
                        s1=phase,
                        imm2=MAGIC,
                    )
                    nc.scalar.activation(
                        cs_groups[t16 // CSG][:, ts(t16 % CSG, N)],
                        d[:],
                        mybir.ActivationFunctionType.Sin,
                        bias=0.0,
                        scale=TWO_PI,
                    )
                return cs_groups

            def cs_chunk(cs_groups, kc, lo, hi):
                return cs_groups[kc // CSG][:, (kc % CSG) * N + lo : (kc % CSG) * N + hi]

            def emit_ff(i, cs):
                ff = ffpool.tile([128, NT * OUT], F32, name=f"ff_{i}", tag="ff")
                for t8 in range(NT):
                    pf = pffp.tile([128, OUT], F32, name=f"pf_{i}_{t8}", tag="pf")
                    for kc in range(KT):
                        nc.tensor.matmul(
                            pf[:],
                            lhsT=cs_chunk(cs, kc, t8 * 128, (t8 + 1) * 128),
                            rhs=w_sb[:, ts(kc, OUT)],
                            start=(kc == 0),
                            stop=(kc == KT - 1),
                        )
                    nc.vector.tensor_copy(ff[:, ts(t8, OUT)], pf[:])
                return ff

            def emit_main(i, ff, a_tiles):
                b = items[i]
                pms = [
                    pmp.tile([128, 2 * OUT], F32, name=f"pm_{i}_{p}", tag="pm")
                    for p in range(NT // 2)
                ]
                for pair in range(NT // 2):
                    nc.tensor.matmul(
                        pms[pair][:],
                        lhsT=ones_sb[:],
                        rhs=b2_sb[:],
                        start=True,
                        stop=False,
                    )
                for jc in range(NT):
                    for pair in range(NT // 2):
                        for half in range(2):
                            it = pair * 2 + half
                            nc.tensor.matmul(
                                pms[pair][:, ts(half, OUT)],
                                lhsT=a_tiles[jc][:, ts(it, 128)],
                                rhs=ff[:, ts(jc, OUT)],
                                start=False,
                                stop=(jc == NT - 1),
                            )
                for pair in range(NT // 2):
                    o_sb = opool.tile(
                        [128, 2 * OUT], F32, name=f"o_{i}_{pair}", tag="o"
                    )
                    nc.vector.tensor_copy(o_sb[:], pms[pair][:])
                    nc.scalar.dma_start(
                        Od[b, pair * 256 : (pair + 1) * 256, :].rearrange(
                            "(h p) o -> p h o", p=128
                        ),
                        o_sb[:].rearrange("p (h o) -> p h o", h=2),
                    )

            def emit_pipeline():
                # software pipeline: prep(i+1), cs(i+1) and the A-load of i+1
                # are emitted before main(i); ff(i+1) right after main(i).
                v2_0 = emit_prep(0)
                cs_0 = emit_cs(0, v2_0)
                ff_cur = emit_ff(0, cs_0)
                a_cur = emit_a_load(0)
                for i in range(L):
                    if i + 1 < L:
                        v2_next = emit_prep(i + 1)
                        cs_next = emit_cs(i + 1, v2_next)
                        a_next = emit_a_load(i + 1)
                    emit_main(i, ff_cur, a_cur)
                    if i + 1 < L:
                        ff_cur = emit_ff(i + 1, cs_next)
                        a_cur = a_next

            if not nc_done:
                if loop:
                    with tc.For_i(0, reps, 1):
                        emit_pipeline()
                else:
                    emit_pipeline()

    nc.finalize()
    _NC_CACHE[key] = nc
    return nc


def make_const_inputs(fouriercoeffs, bias):
    W = np.ascontiguousarray(
        np.asarray(fouriercoeffs, np.float32).transpose(0, 3, 2, 1).reshape(
            2 * G * IN, OUT
        )
    )
    p = np.arange(128)
    gp = np.arange(G // 2)
    # k_g = g+1, g = 2*gp + p//64
    kscale = ((2 * gp[None, :] + p[:, None] // IN + 1) / (2.0 * np.pi)).astype(
        np.float32
    )
    ident = np.eye(128, dtype=np.float32)
    bias2 = np.tile(np.asarray(bias, np.float32).reshape(1, OUT), (1, 2))
    return W, kscale, ident, bias2


def kernel(V, A, fouriercoeffs, bias):
    nc = build_nc()
    W, kscale, ident, bias2 = make_const_inputs(fouriercoeffs, bias)
    V = np.asarray(V, np.float32)
    A = np.asarray(A, np.float32)
    in_maps = []
    for core in range(N_CORES):
        sl = slice(core * B_LOC, (core + 1) * B_LOC)
        in_maps.append(
            {
                "V": np.ascontiguousarray(V[sl]),
                "A": np.ascontiguousarray(A[sl]),
                "W": W,
                "kscale": kscale,
                "ident": ident,
                "bias2": bias2,
            }
        )
    res = run_bass_kernel_spmd(nc, in_maps, list(range(N_CORES)))
    return np.concatenate(
        [res.results[i]["out"] for i in range(N_CORES)], axis=0
    ).astype(np.float32)



# revision 26
# speedup vs baseline: 8.0542x; 8.0542x over previous
"""NaiveFourierKANLayer Trainium2 kernel (8-core SPMD, data-parallel over batch).

Math (per batch b):
  ff[n,o]  = sum_{d,c,g} trig_d(V[n,c]*k_g) * coeffs[d,o,c,g]   (k_g = g+1)
  out[i,o] = sum_{c,j} A[j,c,i] * ff[j,o] + bias[o]

Per core (2 batches/core), the pipeline is software-pipelined so the prep of
batch i+1 (V transpose -> range-reduced cos/sin features -> ff matmul) runs
on DVE/ACT/PE while the A-stream of batch i (the 16 MB/batch HBM roofline
term) is in flight.

  - cs features live as cs^T [2048, 1024] bf16: 16 tiles of [128, 1024];
    partition p of tile t is contraction row t*128+p, row order (d, g, c),
    c fastest. Range reduction (|kx| reaches ~80; the ACT Sin spline is only
    accurate on [-pi, pi]) is one fused custom DVE op per tile:
    d = t - round(t), t = V*(k/2pi) + phase, round() via the fp32
    magic-number trick; then ACT Sin(2pi*d) written as bf16.
  - ff = cs^T.T @ W on PE in bf16 (16 K-chunks), psum fp32 [n=128, o=128],
    copied to SBUF as bf16.
  - A streams as 8 contiguous 2 MB cast-DMAs per batch (SWDGE fp32->bf16,
    16 KB/partition): tile jc = [128 j, (c n)] bf16. No DMA accumulate --
    the c-reduction is folded into the PE contraction instead (4x more
    bf16 matmuls, each ~4x faster than the fp32 ones they replace), which
    removes the SBUF read-modify-write traffic of accumulating DMAs.
  - main matmul: out[i,o] = sum_{(jc,c)} A_chunk.T @ ff[jc], 256 bf16
    matmuls of 128^3 per batch accumulated in one fp32 PSUM tile
    [128, (it o)] spanning 2 banks. PSUM start=True zeroes a whole 2 KB
    zero region (= bank), so a bank-wide rank-1 bias matmul
    (ones[1,128].T @ bias2[1,512]) opens each bank and only the last
    matmul per bank carries stop=True. ACT (Copy, the raw passthrough --
    Identity is a table fn) evicts PSUM, one 512 KB output DMA per batch.

bf16 error budget: rel ~1e-3 on the output (incoherent rounding over the
4096-long contraction), vs the 2e-2 gate.
"""

import numpy as np

import concourse.bacc as bacc
import concourse.tile as tile
from concourse import mybir
from concourse.bass import ts
from concourse.bass_utils import run_bass_kernel_spmd

B, N, C, IN, OUT, G = 16, 1024, 4, 64, 128, 16
N_CORES = 8
B_LOC = B // N_CORES
MAGIC = 12582912.0  # 1.5 * 2**23 : fp32 round-to-nearest-integer magic
TWO_PI = float(2.0 * np.pi)
F32 = mybir.dt.float32
BF16 = mybir.dt.bfloat16

NT = N // 128  # 8 n-tiles / j-chunks / i-tiles
KT = 2 * G * IN // 128  # 16 contraction chunks for the ff matmul


def _register_frac_op():
    """out = t - round(t), t = in0*s0 + s1; round via (t+MAGIC)-MAGIC."""
    import concourse.dve_ops as dvo
    from concourse.dve_spec import Spec, Src0, C0, C1, C2, lower
    from concourse.dve_uop import DveOpSpec

    name = "FRAC_KAN_ANT"
    for op in dvo.OPS:
        if op.name == name:
            return op

    def _ref(in0, in1, s0, s1, imm2):
        t = (np.float32(in0) * np.float32(s0) + np.float32(s1)).astype(np.float32)
        n = ((t + np.float32(imm2)).astype(np.float32) - np.float32(imm2)).astype(
            np.float32
        )
        return (t - n).astype(np.float32)

    t = Src0 * C0 + C1
    n = (t + C2) - C2
    spec = Spec(body=t - n, reference=_ref)
    placeholder = dvo.DveOp(name, spec, subdim=False, uops_sha={})
    dvo.OPS.append(placeholder)
    dvo._SUB_OPCODE_FOR_NAME[name] = dvo._CUSTOM_DVE_ROW_BASE + len(dvo.OPS) - 1
    dvo.CUSTOM_DVE_SPECS[name] = spec
    shas = {}
    for ver in ("v3", "v4"):
        try:
            ds = DveOpSpec(
                name=name,
                opcode=dvo.get_dve_sub_opcode(name),
                uops=lower(spec, ver=ver),
                rd1_en=False,
            )
            shas[ver] = ds.sha(ver)
        except Exception:
            pass
    final = dvo.DveOp(name, spec, subdim=False, uops_sha=shas)
    dvo.OPS[-1] = final
    return final


_NC_CACHE = {}

def build_nc(reps=1, mode="full", loop=False):
    key = (reps, mode, loop)
    if key in _NC_CACHE:
        return _NC_CACHE[key]
    frac_op = _register_frac_op()

    nc = bacc.Bacc("TRN2", target_bir_lowering=False, debug=False)
    Vd = nc.dram_tensor("V", [B_LOC, N, IN], F32, kind="ExternalInput")
    Ad = nc.dram_tensor("A", [B_LOC, N, C, N], F32, kind="ExternalInput")
    Wd = nc.dram_tensor("W", [2 * G * IN, OUT], BF16, kind="ExternalInput")
    KSd = nc.dram_tensor("kscale", [128, G // 2], F32, kind="ExternalInput")
    IDd = nc.dram_tensor("ident", [128, 128], F32, kind="ExternalInput")
    B2d = nc.dram_tensor("bias2", [1, 4 * OUT], BF16, kind="ExternalInput")
    Od = nc.dram_tensor("out", [B_LOC, N, OUT], F32, kind="ExternalOutput")

    if loop:
        items = list(range(B_LOC))  # one pass per For_i iteration
    else:
        items = [b for _ in range(reps) for b in range(B_LOC)]
    L = len(items)

    with tile.TileContext(nc) as tc:
        with (
            tc.tile_pool(name="const", bufs=1) as constp,
            tc.tile_pool(name="v", bufs=2) as vpool,
            tc.tile_pool(name="v2", bufs=2) as v2pool,
            tc.tile_pool(name="d", bufs=2) as dpool,
            tc.tile_pool(name="cs", bufs=1) as cspool,
            tc.tile_pool(name="ff", bufs=2) as ffpool,
            tc.tile_pool(name="a", bufs=16) as apool,
            tc.tile_pool(name="o", bufs=2) as opool,
            tc.tile_pool(name="pm", bufs=2, space="PSUM") as pmp,
            tc.tile_pool(name="pff", bufs=1, space="PSUM") as pffp,
            tc.tile_pool(name="ptr", bufs=2, space="PSUM") as ptrp,
        ):
            # Consts ride the ACT HWDGE ring (identity first -- the
            # V-transposes need it early) so the V load on the SP ring,
            # which feeds the compute-prep chain immediately, isn't queued
            # behind the 0.5 MB W transfer.
            id_sb = constp.tile([128, 128], F32)
            nc.scalar.dma_start(id_sb[:], IDd[:])
            w_sb = constp.tile([128, KT * OUT], BF16)
            nc.scalar.dma_start(
                w_sb[:].rearrange("p (t o) -> p t o", t=KT),
                Wd.rearrange("(t p) o -> p t o", p=128),
            )
            ks_sb = constp.tile([128, G // 2], F32)
            nc.sync.dma_start(ks_sb[:], KSd[:])
            b2_sb = constp.tile([1, 4 * OUT], BF16)
            nc.scalar.dma_start(b2_sb[:], B2d[:])
            ones_sb = constp.tile([1, OUT], BF16)
            nc.vector.memset(ones_sb[:], 1.0)

            def emit_a_load(i):
                """A tiles for item i: 8 contiguous 2 MB cast-DMAs (fp32 in
                HBM -> bf16 in SBUF), tile jc = [128 j, (c n)]."""
                b = items[i]
                a_src = Ad[b].rearrange("(t p) c n -> t p (c n)", p=128)
                a_tiles = [
                    apool.tile([128, C * N], BF16, name=f"a_{i}_{jc}", tag="a")
                    for jc in range(NT)
                ]
                for jc in range(NT):
                    nc.gpsimd.dma_start(a_tiles[jc][:], a_src[jc])
                return a_tiles

            if mode == "dma":
                # A-stream-only floor: same DMA traffic, no compute.
                acc = constp.tile([128, 16], F32)

                def emit_dma_floor():
                    for i, b in enumerate(items):
                        a_tiles = emit_a_load(i)
                        for jc in range(NT):
                            col = (i * NT + jc) % 16
                            nc.vector.reduce_sum(
                                acc[:, col : col + 1],
                                a_tiles[jc][:, 0:512],
                                axis=mybir.AxisListType.X,
                            )

                if loop:
                    with tc.For_i(0, reps, 1):
                        emit_dma_floor()
                else:
                    emit_dma_floor()
                nc_done = True
            else:
                nc_done = False

            def emit_prep(i):
                """V load + transpose + frac/sin features + cs for item i."""
                b = items[i]
                v_sb = vpool.tile([128, NT * IN], F32, name=f"v_{i}", tag="v")
                nc.sync.dma_start(
                    v_sb[:].rearrange("p (t c) -> p t c", t=NT),
                    Vd[b].rearrange("(t p) c -> p t c", p=128),
                )
                v2 = v2pool.tile([128, N], F32, name=f"v2_{i}", tag="v2")
                for t8 in range(NT):
                    ptr = ptrp.tile([IN, 128], F32, name=f"ptr_{i}_{t8}", tag="ptr")
                    nc.tensor.transpose(ptr[:], v_sb[:, ts(t8, IN)], id_sb[:])
                    nc.vector.tensor_copy(v2[0:IN, ts(t8, 128)], ptr[:])
                    nc.vector.tensor_copy(v2[IN : 2 * IN, ts(t8, 128)], ptr[:])
                return v2

            CSG = 4  # K-chunks per cs tile: Tile deps are tile-granular, so
            # smaller cs tiles let ff matmuls start before ALL sins finish.

            def emit_cs(i, v2):
                cs_groups = [
                    cspool.tile(
                        [128, CSG * N], BF16, name=f"cs_{i}_{g}", tag=f"cs{g}"
                    )
                    for g in range(KT // CSG)
                ]
                for t16 in range(KT):
                    gp = t16 % NT
                    phase = 0.25 if t16 < 8 else 0.0  # tiles 0..7 = cos
                    d = dpool.tile([128, N], F32, name=f"d_{i}_{t16}", tag="d")
                    nc.vector._custom_dve(
                        frac_op,
                        out=d[:],
                        in0=v2[:],
                        s0=ks_sb[:, gp : gp + 1],
                        s1=phase,
                        imm2=MAGIC,
                    )
                    nc.scalar.activation(
                        cs_groups[t16 // CSG][:, ts(t16 % CSG, N)],
                        d[:],
                        mybir.ActivationFunctionType.Sin,
                        bias=0.0,
                        scale=TWO_PI,
                    )
                return cs_groups

            def cs_chunk(cs_groups, kc, lo, hi):
                return cs_groups[kc // CSG][:, (kc % CSG) * N + lo : (kc % CSG) * N + hi]

            def emit_ff(i, cs):
                """kc-outer order: the first matmuls only need cs group 0,
                so ff overlaps the sin stream instead of waiting for all 16
                feature tiles. 8 interleaved PSUM accumulation groups live
                in one [128, NT*OUT] tile; ACT evicts it (DVE is busier)."""
                ff = ffpool.tile([128, NT * OUT], BF16, name=f"ff_{i}", tag="ff")
                pf = pffp.tile([128, NT * OUT], F32, name=f"pf_{i}", tag="pf")
                # PSUM start=True zeroes/claims a whole 2 KB zero region (one
                # bank = 4 of these [128,128] f32 regions): only the first MM
                # touching each bank opens it, only the last closes it.
                for kc in range(KT):
                    for t8 in range(NT):
                        nc.tensor.matmul(
                            pf[:, ts(t8, OUT)],
                            lhsT=cs_chunk(cs, kc, t8 * 128, (t8 + 1) * 128),
                            rhs=w_sb[:, ts(kc, OUT)],
                            start=(kc == 0 and t8 % 4 == 0),
                            stop=(kc == KT - 1 and t8 % 4 == 3),
                        )
                nc.scalar.activation(
                    ff[:], pf[:], mybir.ActivationFunctionType.Copy
                )
                return ff

            def emit_main(i, ff, a_tiles):
                """One PSUM tile [128, (it o)] = 4 KB = 2 banks holds the
                whole batch output. A bank-wide bias matmul opens (zeroes)
                each bank; the last matmul per bank closes it. ACT evicts,
                one 512 KB output DMA."""
                b = items[i]
                pm = pmp.tile([128, NT * OUT], F32, name=f"pm_{i}", tag="pm")
                for bank in range(2):
                    nc.tensor.matmul(
                        pm[:, bank * 4 * OUT : (bank + 1) * 4 * OUT],
                        lhsT=ones_sb[:],
                        rhs=b2_sb[:],
                        start=True,
                        stop=False,
                    )
                for jc in range(NT):
                    for c in range(C):
                        for it in range(NT):
                            last = jc == NT - 1 and c == C - 1
                            nc.tensor.matmul(
                                pm[:, ts(it, OUT)],
                                lhsT=a_tiles[jc][
                                    :, c * N + it * 128 : c * N + (it + 1) * 128
                                ],
                                rhs=ff[:, ts(jc, OUT)],
                                start=False,
                                stop=(last and it % 4 == 3),
                            )
                o_sb = opool.tile([128, NT * OUT], F32, name=f"o_{i}", tag="o")
                nc.scalar.activation(
                    o_sb[:], pm[:], mybir.ActivationFunctionType.Copy
                )
                nc.scalar.dma_start(
                    Od[b].rearrange("(t p) o -> p t o", p=128),
                    o_sb[:].rearrange("p (t o) -> p t o", t=NT),
                )

            def emit_pipeline():
                # software pipeline: prep(i+1), cs(i+1) and the A-load of i+1
                # are emitted before main(i); ff(i+1) right after main(i).
                v2_0 = emit_prep(0)
                cs_0 = emit_cs(0, v2_0)
                ff_cur = emit_ff(0, cs_0)
                a_cur = emit_a_load(0)
                for i in range(L):
                    if i + 1 < L:
                        a_next = emit_a_load(i + 1)  # first: SWDGE queue order
                        v2_next = emit_prep(i + 1)
                        cs_next = emit_cs(i + 1, v2_next)
                    emit_main(i, ff_cur, a_cur)
                    if i + 1 < L:
                        ff_cur = emit_ff(i + 1, cs_next)
                        a_cur = a_next

            if not nc_done:
                if loop:
                    with tc.For_i(0, reps, 1):
                        emit_pipeline()
                else:
                    emit_pipeline()

    nc.finalize()
    _NC_CACHE[key] = nc
    return nc


def make_const_inputs(fouriercoeffs, bias):
    import ml_dtypes

    W = np.ascontiguousarray(
        np.asarray(fouriercoeffs, np.float32)
        .transpose(0, 3, 2, 1)
        .reshape(2 * G * IN, OUT)
    ).astype(ml_dtypes.bfloat16)
    p = np.arange(128)
    gp = np.arange(G // 2)
    # k_g = g+1, g = 2*gp + p//64
    kscale = ((2 * gp[None, :] + p[:, None] // IN + 1) / (2.0 * np.pi)).astype(
        np.float32
    )
    ident = np.eye(128, dtype=np.float32)
    bias2 = np.tile(np.asarray(bias, np.float32).reshape(1, OUT), (1, 4)).astype(
        ml_dtypes.bfloat16
    )
    return W, kscale, ident, bias2


def kernel(V, A, fouriercoeffs, bias):
    nc = build_nc()
    W, kscale, ident, bias2 = make_const_inputs(fouriercoeffs, bias)
    V = np.asarray(V, np.float32)
    A = np.asarray(A, np.float32)
    in_maps = []
    for core in range(N_CORES):
        sl = slice(core * B_LOC, (core + 1) * B_LOC)
        in_maps.append(
            {
                "V": np.ascontiguousarray(V[sl]),
                "A": np.ascontiguousarray(A[sl]),
                "W": W,
                "kscale": kscale,
                "ident": ident,
                "bias2": bias2,
            }
        )
    res = run_bass_kernel_spmd(nc, in_maps, list(range(N_CORES)))
    return np.concatenate(
        [res.results[i]["out"] for i in range(N_CORES)], axis=0
    ).astype(np.float32)


# revision 30
# speedup vs baseline: 8.4614x; 1.0506x over previous
"""NaiveFourierKANLayer Trainium2 kernel (8-core SPMD, data-parallel over batch).

Math (per batch b):
  ff[n,o]  = sum_{d,c,g} trig_d(V[n,c]*k_g) * coeffs[d,o,c,g]   (k_g = g+1)
  out[i,o] = sum_{c,j} A[j,c,i] * ff[j,o] + bias[o]

Per core (2 batches/core), the pipeline is software-pipelined so the prep of
batch i+1 (V transpose -> range-reduced cos/sin features -> ff matmul) runs
on DVE/ACT/PE while the A-stream of batch i (the 16 MB/batch HBM roofline
term) is in flight.

  - cs features live as cs^T [2048, 1024] bf16: 16 tiles of [128, 1024];
    partition p of tile t is contraction row t*128+p, row order (d, g, c),
    c fastest. Range reduction (|kx| reaches ~80; the ACT Sin spline is only
    accurate on [-pi, pi]) is one fused custom DVE op per tile:
    d = t - round(t), t = V*(k/2pi) + phase, round() via the fp32
    magic-number trick; then ACT Sin(2pi*d) written as bf16.
  - ff = cs^T.T @ W on PE in bf16 (16 K-chunks), psum fp32 [n=128, o=128],
    copied to SBUF as bf16.
  - A streams as 16 contiguous 1 MB-read cast-DMAs per batch (SWDGE
    fp32->bf16, 8 KB/partition): tile jc = [128 j, (c n)] bf16, two halves
    per tile -- finer DMA grain overlaps per-transfer completion-latency
    tails (measured ~9 us/pass faster than 2 MB transfers). No DMA accum --
    the c-reduction is folded into the PE contraction instead (4x more
    bf16 matmuls, each ~4x faster than the fp32 ones they replace), which
    removes the SBUF read-modify-write traffic of accumulating DMAs.
  - main matmul: out[i,o] = sum_{(jc,c)} A_chunk.T @ ff[jc], 256 bf16
    matmuls of 128^3 per batch accumulated in one fp32 PSUM tile
    [128, (it o)] spanning 2 banks. PSUM start=True zeroes a whole 2 KB
    zero region (= bank), so a bank-wide rank-1 bias matmul
    (ones[1,128].T @ bias2[1,512]) opens each bank and only the last
    matmul per bank carries stop=True. ACT (Copy, the raw passthrough --
    Identity is a table fn) evicts PSUM, one 512 KB output DMA per batch.

bf16 error budget: rel ~1e-3 on the output (incoherent rounding over the
4096-long contraction), vs the 2e-2 gate.
"""

import numpy as np

import concourse.bacc as bacc
import concourse.tile as tile
from concourse import mybir
from concourse.bass import ts
from concourse.bass_utils import run_bass_kernel_spmd

B, N, C, IN, OUT, G = 16, 1024, 4, 64, 128, 16
N_CORES = 8
B_LOC = B // N_CORES
MAGIC = 12582912.0  # 1.5 * 2**23 : fp32 round-to-nearest-integer magic
TWO_PI = float(2.0 * np.pi)
F32 = mybir.dt.float32
BF16 = mybir.dt.bfloat16

NT = N // 128  # 8 n-tiles / j-chunks / i-tiles
KT = 2 * G * IN // 128  # 16 contraction chunks for the ff matmul


def _register_frac_op():
    """out = t - round(t), t = in0*s0 + s1; round via (t+MAGIC)-MAGIC."""
    import concourse.dve_ops as dvo
    from concourse.dve_spec import Spec, Src0, C0, C1, C2, lower
    from concourse.dve_uop import DveOpSpec

    name = "FRAC_KAN_ANT"
    for op in dvo.OPS:
        if op.name == name:
            return op

    def _ref(in0, in1, s0, s1, imm2):
        t = (np.float32(in0) * np.float32(s0) + np.float32(s1)).astype(np.float32)
        n = ((t + np.float32(imm2)).astype(np.float32) - np.float32(imm2)).astype(
            np.float32
        )
        return (t - n).astype(np.float32)

    t = Src0 * C0 + C1
    n = (t + C2) - C2
    spec = Spec(body=t - n, reference=_ref)
    placeholder = dvo.DveOp(name, spec, subdim=False, uops_sha={})
    dvo.OPS.append(placeholder)
    dvo._SUB_OPCODE_FOR_NAME[name] = dvo._CUSTOM_DVE_ROW_BASE + len(dvo.OPS) - 1
    dvo.CUSTOM_DVE_SPECS[name] = spec
    shas = {}
    for ver in ("v3", "v4"):
        try:
            ds = DveOpSpec(
                name=name,
                opcode=dvo.get_dve_sub_opcode(name),
                uops=lower(spec, ver=ver),
                rd1_en=False,
            )
            shas[ver] = ds.sha(ver)
        except Exception:
            pass
    final = dvo.DveOp(name, spec, subdim=False, uops_sha=shas)
    dvo.OPS[-1] = final
    return final


_NC_CACHE = {}

def build_nc(reps=1, mode="full", loop=False):
    key = (reps, mode, loop)
    if key in _NC_CACHE:
        return _NC_CACHE[key]
    frac_op = _register_frac_op()

    nc = bacc.Bacc("TRN2", target_bir_lowering=False, debug=False)
    Vd = nc.dram_tensor("V", [B_LOC, N, IN], F32, kind="ExternalInput")
    Ad = nc.dram_tensor("A", [B_LOC, N, C, N], F32, kind="ExternalInput")
    Wd = nc.dram_tensor("W", [2 * G * IN, OUT], BF16, kind="ExternalInput")
    KSd = nc.dram_tensor("kscale", [128, G // 2], F32, kind="ExternalInput")
    IDd = nc.dram_tensor("ident", [128, 128], F32, kind="ExternalInput")
    B2d = nc.dram_tensor("bias2", [1, 4 * OUT], BF16, kind="ExternalInput")
    Od = nc.dram_tensor("out", [B_LOC, N, OUT], F32, kind="ExternalOutput")

    if loop:
        items = list(range(B_LOC))  # one pass per For_i iteration
    else:
        items = [b for _ in range(reps) for b in range(B_LOC)]
    L = len(items)

    with tile.TileContext(nc) as tc:
        with (
            tc.tile_pool(name="const", bufs=1) as constp,
            tc.tile_pool(name="v", bufs=2) as vpool,
            tc.tile_pool(name="v2", bufs=2) as v2pool,
            tc.tile_pool(name="d", bufs=2) as dpool,
            tc.tile_pool(name="cs", bufs=1) as cspool,
            tc.tile_pool(name="ff", bufs=2) as ffpool,
            tc.tile_pool(name="a", bufs=16) as apool,
            tc.tile_pool(name="o", bufs=2) as opool,
            tc.tile_pool(name="pm", bufs=2, space="PSUM") as pmp,
            tc.tile_pool(name="pff", bufs=1, space="PSUM") as pffp,
            tc.tile_pool(name="ptr", bufs=2, space="PSUM") as ptrp,
        ):
            # Consts ride the ACT HWDGE ring (identity first -- the
            # V-transposes need it early) so the V load on the SP ring,
            # which feeds the compute-prep chain immediately, isn't queued
            # behind the 0.5 MB W transfer.
            id_sb = constp.tile([128, 128], F32)
            nc.scalar.dma_start(id_sb[:], IDd[:])
            w_sb = constp.tile([128, KT * OUT], BF16)
            nc.scalar.dma_start(
                w_sb[:].rearrange("p (t o) -> p t o", t=KT),
                Wd.rearrange("(t p) o -> p t o", p=128),
            )
            ks_sb = constp.tile([128, G // 2], F32)
            nc.sync.dma_start(ks_sb[:], KSd[:])
            b2_sb = constp.tile([1, 4 * OUT], BF16)
            nc.scalar.dma_start(b2_sb[:], B2d[:])
            ones_sb = constp.tile([1, OUT], BF16)
            nc.vector.memset(ones_sb[:], 1.0)

            def emit_a_load(i):
                """A tiles for item i: cast-DMAs (fp32 in HBM -> bf16 in
                SBUF), tile jc = [128 j, (c n)]. Each tile loads as two
                1 MB-read halves: finer grain smooths round-robin with the
                V/output DMAs and frees/claims ring buffers sooner."""
                b = items[i]
                a_src = Ad[b].rearrange("(t p) c n -> t p (c n)", p=128)
                a_tiles = [
                    apool.tile([128, C * N], BF16, name=f"a_{i}_{jc}", tag="a")
                    for jc in range(NT)
                ]
                half = C * N // 2
                for jc in range(NT):
                    for h in range(2):
                        nc.gpsimd.dma_start(
                            a_tiles[jc][:, h * half : (h + 1) * half],
                            a_src[jc][:, h * half : (h + 1) * half],
                        )
                return a_tiles

            if mode == "dma":
                # A-stream-only floor: same DMA traffic, no compute.
                acc = constp.tile([128, 16], F32)

                def emit_dma_floor():
                    for i, b in enumerate(items):
                        a_tiles = emit_a_load(i)
                        for jc in range(NT):
                            col = (i * NT + jc) % 16
                            nc.vector.reduce_sum(
                                acc[:, col : col + 1],
                                a_tiles[jc][:, 0:512],
                                axis=mybir.AxisListType.X,
                            )

                if loop:
                    with tc.For_i(0, reps, 1):
                        emit_dma_floor()
                else:
                    emit_dma_floor()
                nc_done = True
            else:
                nc_done = False

            def emit_prep(i):
                """V load + transpose + frac/sin features + cs for item i."""
                b = items[i]
                v_sb = vpool.tile([128, NT * IN], F32, name=f"v_{i}", tag="v")
                nc.sync.dma_start(
                    v_sb[:].rearrange("p (t c) -> p t c", t=NT),
                    Vd[b].rearrange("(t p) c -> p t c", p=128),
                )
                v2 = v2pool.tile([128, N], F32, name=f"v2_{i}", tag="v2")
                for t8 in range(NT):
                    ptr = ptrp.tile([IN, 128], F32, name=f"ptr_{i}_{t8}", tag="ptr")
                    nc.tensor.transpose(ptr[:], v_sb[:, ts(t8, IN)], id_sb[:])
                    nc.vector.tensor_copy(v2[0:IN, ts(t8, 128)], ptr[:])
                    nc.vector.tensor_copy(v2[IN : 2 * IN, ts(t8, 128)], ptr[:])
                return v2

            CSG = 4  # K-chunks per cs tile: Tile deps are tile-granular, so
            # smaller cs tiles let ff matmuls start before ALL sins finish.

            def emit_cs(i, v2):
                cs_groups = [
                    cspool.tile(
                        [128, CSG * N], BF16, name=f"cs_{i}_{g}", tag=f"cs{g}"
                    )
                    for g in range(KT // CSG)
                ]
                for t16 in range(KT):
                    gp = t16 % NT
                    phase = 0.25 if t16 < 8 else 0.0  # tiles 0..7 = cos
                    d = dpool.tile([128, N], F32, name=f"d_{i}_{t16}", tag="d")
                    nc.vector._custom_dve(
                        frac_op,
                        out=d[:],
                        in0=v2[:],
                        s0=ks_sb[:, gp : gp + 1],
                        s1=phase,
                        imm2=MAGIC,
                    )
                    nc.scalar.activation(
                        cs_groups[t16 // CSG][:, ts(t16 % CSG, N)],
                        d[:],
                        mybir.ActivationFunctionType.Sin,
                        bias=0.0,
                        scale=TWO_PI,
                    )
                return cs_groups

            def cs_chunk(cs_groups, kc, lo, hi):
                return cs_groups[kc // CSG][:, (kc % CSG) * N + lo : (kc % CSG) * N + hi]

            def emit_ff(i, cs):
                """kc-outer order: the first matmuls only need cs group 0,
                so ff overlaps the sin stream instead of waiting for all 16
                feature tiles. 8 interleaved PSUM accumulation groups live
                in one [128, NT*OUT] tile; ACT evicts it (DVE is busier)."""
                ff = ffpool.tile([128, NT * OUT], BF16, name=f"ff_{i}", tag="ff")
                pf = pffp.tile([128, NT * OUT], F32, name=f"pf_{i}", tag="pf")
                # PSUM start=True zeroes/claims a whole 2 KB zero region (one
                # bank = 4 of these [128,128] f32 regions): only the first MM
                # touching each bank opens it, only the last closes it.
                for kc in range(KT):
                    for t8 in range(NT):
                        nc.tensor.matmul(
                            pf[:, ts(t8, OUT)],
                            lhsT=cs_chunk(cs, kc, t8 * 128, (t8 + 1) * 128),
                            rhs=w_sb[:, ts(kc, OUT)],
                            start=(kc == 0 and t8 % 4 == 0),
                            stop=(kc == KT - 1 and t8 % 4 == 3),
                        )
                nc.scalar.activation(
                    ff[:], pf[:], mybir.ActivationFunctionType.Copy
                )
                return ff

            def emit_main(i, ff, a_tiles):
                """One PSUM tile [128, (it o)] = 4 KB = 2 banks holds the
                whole batch output. A bank-wide bias matmul opens (zeroes)
                each bank; the last matmul per bank closes it. ACT evicts,
                one 512 KB output DMA."""
                b = items[i]
                pm = pmp.tile([128, NT * OUT], F32, name=f"pm_{i}", tag="pm")
                for bank in range(2):
                    nc.tensor.matmul(
                        pm[:, bank * 4 * OUT : (bank + 1) * 4 * OUT],
                        lhsT=ones_sb[:],
                        rhs=b2_sb[:],
                        start=True,
                        stop=False,
                    )
                for jc in range(NT):
                    for c in range(C):
                        for it in range(NT):
                            last = jc == NT - 1 and c == C - 1
                            nc.tensor.matmul(
                                pm[:, ts(it, OUT)],
                                lhsT=a_tiles[jc][
                                    :, c * N + it * 128 : c * N + (it + 1) * 128
                                ],
                                rhs=ff[:, ts(jc, OUT)],
                                start=False,
                                stop=(last and it % 4 == 3),
                            )
                o_sb = opool.tile([128, NT * OUT], F32, name=f"o_{i}", tag="o")
                nc.scalar.activation(
                    o_sb[:], pm[:], mybir.ActivationFunctionType.Copy
                )
                # SP ring: the ACT ring carries the W/id const loads.
                nc.sync.dma_start(
                    Od[b].rearrange("(t p) o -> p t o", p=128),
                    o_sb[:].rearrange("p (t o) -> p t o", t=NT),
                )

            def emit_pipeline():
                # software pipeline: prep(i+1), cs(i+1) and the A-load of i+1
                # are emitted before main(i); ff(i+1) right after main(i).
                v2_0 = emit_prep(0)
                cs_0 = emit_cs(0, v2_0)
                ff_cur = emit_ff(0, cs_0)
                a_cur = emit_a_load(0)
                for i in range(L):
                    if i + 1 < L:
                        a_next = emit_a_load(i + 1)  # first: SWDGE queue order
                        v2_next = emit_prep(i + 1)
                        cs_next = emit_cs(i + 1, v2_next)
                    emit_main(i, ff_cur, a_cur)
                    if i + 1 < L:
                        ff_cur = emit_ff(i + 1, cs_next)
                        a_cur = a_next

            if not nc_done:
                if loop:
                    with tc.For_i(0, reps, 1):
                        emit_pipeline()
                else:
                    emit_pipeline()

    nc.finalize()
    _NC_CACHE[key] = nc
    return nc


def make_const_inputs(fouriercoeffs, bias):
    import ml_dtypes

    W = np.ascontiguousarray(
        np.asarray(fouriercoeffs, np.float32)
        .transpose(0, 3, 2, 1)
        .reshape(2 * G * IN, OUT)
    ).astype(ml_dtypes.bfloat16)
    p = np.arange(128)
    gp = np.arange(G // 2)
    # k_g = g+1, g = 2*gp + p//64
    kscale = ((2 * gp[None, :] + p[:, None] // IN + 1) / (2.0 * np.pi)).astype(
        np.float32
    )
    ident = np.eye(128, dtype=np.float32)
    bias2 = np.tile(np.asarray(bias, np.float32).reshape(1, OUT), (1, 4)).astype(
        ml_dtypes.bfloat16
    )
    return W, kscale, ident, bias2


def kernel(V, A, fouriercoeffs, bias):
    nc = build_nc()
    W, kscale, ident, bias2 = make_const_inputs(fouriercoeffs, bias)
    V = np.asarray(V, np.float32)
    A = np.asarray(A, np.float32)
    in_maps = []
    for core in range(N_CORES):
        sl = slice(core * B_LOC, (core + 1) * B_LOC)
        in_maps.append(
            {
                "V": np.ascontiguousarray(V[sl]),
                "A": np.ascontiguousarray(A[sl]),
                "W": W,
                "kscale": kscale,
                "ident": ident,
                "bias2": bias2,
            }
        )
    res = run_bass_kernel_spmd(nc, in_maps, list(range(N_CORES)))
    return np.concatenate(
        [res.results[i]["out"] for i in range(N_CORES)], axis=0
    ).astype(np.float32)


# revision 31
# speedup vs baseline: 8.5713x; 1.0130x over previous
"""NaiveFourierKANLayer Trainium2 kernel (8-core SPMD, data-parallel over batch).

Math (per batch b):
  ff[n,o]  = sum_{d,c,g} trig_d(V[n,c]*k_g) * coeffs[d,o,c,g]   (k_g = g+1)
  out[i,o] = sum_{c,j} A[j,c,i] * ff[j,o] + bias[o]

Per core (2 batches/core), the pipeline is software-pipelined so the prep of
batch i+1 (V transpose -> range-reduced cos/sin features -> ff matmul) runs
on DVE/ACT/PE while the A-stream of batch i (the 16 MB/batch HBM roofline
term) is in flight.

  - cs features live as cs^T [2048, 1024] bf16: 16 tiles of [128, 1024];
    partition p of tile t is contraction row t*128+p, row order (d, g, c),
    c fastest. Range reduction (|kx| reaches ~80; the ACT Sin spline is only
    accurate on [-pi, pi]) is one fused custom DVE op per tile:
    d = t - round(t), t = V*(k/2pi) + phase, round() via the fp32
    magic-number trick; then ACT Sin(2pi*d) written as bf16.
  - ff = cs^T.T @ W on PE in bf16 (16 K-chunks), psum fp32 [n=128, o=128],
    copied to SBUF as bf16.
  - A streams as 16 contiguous 1 MB-read cast-DMAs per batch (SWDGE
    fp32->bf16, 8 KB/partition): tile jc = [128 j, (c n)] bf16, two halves
    per tile -- finer DMA grain overlaps per-transfer completion-latency
    tails (measured ~9 us/pass faster than 2 MB transfers). No DMA accum --
    the c-reduction is folded into the PE contraction instead (4x more
    bf16 matmuls, each ~4x faster than the fp32 ones they replace), which
    removes the SBUF read-modify-write traffic of accumulating DMAs.
  - main matmul: out[i,o] = sum_{(jc,c)} A_chunk.T @ ff[jc], 256 bf16
    matmuls of 128^3 per batch accumulated in one fp32 PSUM tile
    [128, (it o)] spanning 2 banks. PSUM start=True zeroes a whole 2 KB
    zero region (= bank), so a bank-wide rank-1 bias matmul
    (ones[1,128].T @ bias2[1,512]) opens each bank and only the last
    matmul per bank carries stop=True. ACT (Copy, the raw passthrough --
    Identity is a table fn) evicts PSUM, one 512 KB output DMA per batch.

bf16 error budget: rel ~1e-3 on the output (incoherent rounding over the
4096-long contraction), vs the 2e-2 gate.
"""

import numpy as np

import concourse.bacc as bacc
import concourse.tile as tile
from concourse import mybir
from concourse.bass import ts
from concourse.bass_utils import run_bass_kernel_spmd

B, N, C, IN, OUT, G = 16, 1024, 4, 64, 128, 16
N_CORES = 8
B_LOC = B // N_CORES
MAGIC = 12582912.0  # 1.5 * 2**23 : fp32 round-to-nearest-integer magic
TWO_PI = float(2.0 * np.pi)
F32 = mybir.dt.float32
BF16 = mybir.dt.bfloat16

NT = N // 128  # 8 n-tiles / j-chunks / i-tiles
KT = 2 * G * IN // 128  # 16 contraction chunks for the ff matmul


def _register_frac_op():
    """out = t - round(t), t = in0*s0 + s1; round via (t+MAGIC)-MAGIC."""
    import concourse.dve_ops as dvo
    from concourse.dve_spec import Spec, Src0, C0, C1, C2, lower
    from concourse.dve_uop import DveOpSpec

    name = "FRAC_KAN_ANT"
    for op in dvo.OPS:
        if op.name == name:
            return op

    def _ref(in0, in1, s0, s1, imm2):
        t = (np.float32(in0) * np.float32(s0) + np.float32(s1)).astype(np.float32)
        n = ((t + np.float32(imm2)).astype(np.float32) - np.float32(imm2)).astype(
            np.float32
        )
        return (t - n).astype(np.float32)

    t = Src0 * C0 + C1
    n = (t + C2) - C2
    spec = Spec(body=t - n, reference=_ref)
    placeholder = dvo.DveOp(name, spec, subdim=False, uops_sha={})
    dvo.OPS.append(placeholder)
    dvo._SUB_OPCODE_FOR_NAME[name] = dvo._CUSTOM_DVE_ROW_BASE + len(dvo.OPS) - 1
    dvo.CUSTOM_DVE_SPECS[name] = spec
    shas = {}
    for ver in ("v3", "v4"):
        try:
            ds = DveOpSpec(
                name=name,
                opcode=dvo.get_dve_sub_opcode(name),
                uops=lower(spec, ver=ver),
                rd1_en=False,
            )
            shas[ver] = ds.sha(ver)
        except Exception:
            pass
    final = dvo.DveOp(name, spec, subdim=False, uops_sha=shas)
    dvo.OPS[-1] = final
    return final


_NC_CACHE = {}

def build_nc(reps=1, mode="full", loop=False):
    key = (reps, mode, loop)
    if key in _NC_CACHE:
        return _NC_CACHE[key]
    frac_op = _register_frac_op()

    nc = bacc.Bacc("TRN2", target_bir_lowering=False, debug=False)
    Vd = nc.dram_tensor("V", [B_LOC, N, IN], F32, kind="ExternalInput")
    Ad = nc.dram_tensor("A", [B_LOC, N, C, N], F32, kind="ExternalInput")
    Wd = nc.dram_tensor("W", [2 * G * IN, OUT], BF16, kind="ExternalInput")
    KSd = nc.dram_tensor("kscale", [128, G // 2], F32, kind="ExternalInput")
    IDd = nc.dram_tensor("ident", [128, 128], F32, kind="ExternalInput")
    B2d = nc.dram_tensor("bias2", [1, 4 * OUT], BF16, kind="ExternalInput")
    Od = nc.dram_tensor("out", [B_LOC, N, OUT], F32, kind="ExternalOutput")

    if loop:
        items = list(range(B_LOC))  # one pass per For_i iteration
    else:
        items = [b for _ in range(reps) for b in range(B_LOC)]
    L = len(items)

    with tile.TileContext(nc) as tc:
        with (
            tc.tile_pool(name="const", bufs=1) as constp,
            tc.tile_pool(name="v", bufs=2) as vpool,
            tc.tile_pool(name="v2", bufs=2) as v2pool,
            tc.tile_pool(name="d", bufs=2) as dpool,
            tc.tile_pool(name="cs", bufs=1) as cspool,
            tc.tile_pool(name="ff", bufs=2) as ffpool,
            tc.tile_pool(name="a", bufs=32) as apool,
            tc.tile_pool(name="o", bufs=2) as opool,
            tc.tile_pool(name="pm", bufs=2, space="PSUM") as pmp,
            tc.tile_pool(name="pff", bufs=1, space="PSUM") as pffp,
            tc.tile_pool(name="ptr", bufs=2, space="PSUM") as ptrp,
        ):
            # Consts ride the ACT HWDGE ring (identity first -- the
            # V-transposes need it early) so the V load on the SP ring,
            # which feeds the compute-prep chain immediately, isn't queued
            # behind the 0.5 MB W transfer.
            id_sb = constp.tile([128, 128], F32)
            nc.scalar.dma_start(id_sb[:], IDd[:])
            w_sb = constp.tile([128, KT * OUT], BF16)
            nc.scalar.dma_start(
                w_sb[:].rearrange("p (t o) -> p t o", t=KT),
                Wd.rearrange("(t p) o -> p t o", p=128),
            )
            ks_sb = constp.tile([128, G // 2], F32)
            nc.sync.dma_start(ks_sb[:], KSd[:])
            b2_sb = constp.tile([1, 4 * OUT], BF16)
            nc.scalar.dma_start(b2_sb[:], B2d[:])
            ones_sb = constp.tile([1, OUT], BF16)
            nc.vector.memset(ones_sb[:], 1.0)

            def emit_a_load(i):
                """A tiles for item i: cast-DMAs (fp32 in HBM -> bf16 in
                SBUF). Each j-chunk loads as TWO separate [128 j, 2N] tiles
                (c01 / c23), one 1 MB-read DMA each: the 1 MB grain overlaps
                per-transfer completion tails, and separate tiles make the
                dependency half-granular -- matmuls for c in {0,1} start
                before the c23 half of the chunk has landed."""
                b = items[i]
                a_src = Ad[b].rearrange("(t p) c n -> t p (c n)", p=128)
                half = C * N // 2
                a_tiles = []
                for jc in range(NT):
                    pair = []
                    for h in range(2):
                        t = apool.tile(
                            [128, half], BF16, name=f"a_{i}_{jc}_{h}", tag="a"
                        )
                        nc.gpsimd.dma_start(
                            t[:], a_src[jc][:, h * half : (h + 1) * half]
                        )
                        pair.append(t)
                    a_tiles.append(pair)
                return a_tiles

            if mode == "dma":
                # A-stream-only floor: same DMA traffic, no compute.
                acc = constp.tile([128, 16], F32)

                def emit_dma_floor():
                    for i, b in enumerate(items):
                        a_tiles = emit_a_load(i)
                        for jc in range(NT):
                            col = (i * NT + jc) % 16
                            nc.vector.reduce_sum(
                                acc[:, col : col + 1],
                                a_tiles[jc][0][:, 0:512],
                                axis=mybir.AxisListType.X,
                            )

                if loop:
                    with tc.For_i(0, reps, 1):
                        emit_dma_floor()
                else:
                    emit_dma_floor()
                nc_done = True
            else:
                nc_done = False

            def emit_prep(i):
                """V load + transpose + frac/sin features + cs for item i."""
                b = items[i]
                v_sb = vpool.tile([128, NT * IN], F32, name=f"v_{i}", tag="v")
                nc.sync.dma_start(
                    v_sb[:].rearrange("p (t c) -> p t c", t=NT),
                    Vd[b].rearrange("(t p) c -> p t c", p=128),
                )
                v2 = v2pool.tile([128, N], F32, name=f"v2_{i}", tag="v2")
                for t8 in range(NT):
                    ptr = ptrp.tile([IN, 128], F32, name=f"ptr_{i}_{t8}", tag="ptr")
                    nc.tensor.transpose(ptr[:], v_sb[:, ts(t8, IN)], id_sb[:])
                    nc.vector.tensor_copy(v2[0:IN, ts(t8, 128)], ptr[:])
                    nc.vector.tensor_copy(v2[IN : 2 * IN, ts(t8, 128)], ptr[:])
                return v2

            CSG = 4  # K-chunks per cs tile: Tile deps are tile-granular, so
            # smaller cs tiles let ff matmuls start before ALL sins finish.

            def emit_cs(i, v2):
                cs_groups = [
                    cspool.tile(
                        [128, CSG * N], BF16, name=f"cs_{i}_{g}", tag=f"cs{g}"
                    )
                    for g in range(KT // CSG)
                ]
                for t16 in range(KT):
                    gp = t16 % NT
                    phase = 0.25 if t16 < 8 else 0.0  # tiles 0..7 = cos
                    d = dpool.tile([128, N], F32, name=f"d_{i}_{t16}", tag="d")
                    nc.vector._custom_dve(
                        frac_op,
                        out=d[:],
                        in0=v2[:],
                        s0=ks_sb[:, gp : gp + 1],
                        s1=phase,
                        imm2=MAGIC,
                    )
                    nc.scalar.activation(
                        cs_groups[t16 // CSG][:, ts(t16 % CSG, N)],
                        d[:],
                        mybir.ActivationFunctionType.Sin,
                        bias=0.0,
                        scale=TWO_PI,
                    )
                return cs_groups

            def cs_chunk(cs_groups, kc, lo, hi):
                return cs_groups[kc // CSG][:, (kc % CSG) * N + lo : (kc % CSG) * N + hi]

            def emit_ff(i, cs):
                """kc-outer order: the first matmuls only need cs group 0,
                so ff overlaps the sin stream instead of waiting for all 16
                feature tiles. 8 interleaved PSUM accumulation groups live
                in one [128, NT*OUT] tile; ACT evicts it (DVE is busier)."""
                ff = ffpool.tile([128, NT * OUT], BF16, name=f"ff_{i}", tag="ff")
                pf = pffp.tile([128, NT * OUT], F32, name=f"pf_{i}", tag="pf")
                # PSUM start=True zeroes/claims a whole 2 KB zero region (one
                # bank = 4 of these [128,128] f32 regions): only the first MM
                # touching each bank opens it, only the last closes it.
                for kc in range(KT):
                    for t8 in range(NT):
                        nc.tensor.matmul(
                            pf[:, ts(t8, OUT)],
                            lhsT=cs_chunk(cs, kc, t8 * 128, (t8 + 1) * 128),
                            rhs=w_sb[:, ts(kc, OUT)],
                            start=(kc == 0 and t8 % 4 == 0),
                            stop=(kc == KT - 1 and t8 % 4 == 3),
                        )
                nc.scalar.activation(
                    ff[:], pf[:], mybir.ActivationFunctionType.Copy
                )
                return ff

            def emit_main(i, ff, a_tiles):
                """One PSUM tile [128, (it o)] = 4 KB = 2 banks holds the
                whole batch output. A bank-wide bias matmul opens (zeroes)
                each bank; the last matmul per bank closes it. ACT evicts,
                one 512 KB output DMA."""
                b = items[i]
                pm = pmp.tile([128, NT * OUT], F32, name=f"pm_{i}", tag="pm")
                for bank in range(2):
                    nc.tensor.matmul(
                        pm[:, bank * 4 * OUT : (bank + 1) * 4 * OUT],
                        lhsT=ones_sb[:],
                        rhs=b2_sb[:],
                        start=True,
                        stop=False,
                    )
                for jc in range(NT):
                    for c in range(C):
                        for it in range(NT):
                            last = jc == NT - 1 and c == C - 1
                            nc.tensor.matmul(
                                pm[:, ts(it, OUT)],
                                lhsT=a_tiles[jc][c // 2][
                                    :, (c % 2) * N + it * 128 : (c % 2) * N + (it + 1) * 128
                                ],
                                rhs=ff[:, ts(jc, OUT)],
                                start=False,
                                stop=(last and it % 4 == 3),
                            )
                o_sb = opool.tile([128, NT * OUT], F32, name=f"o_{i}", tag="o")
                nc.scalar.activation(
                    o_sb[:], pm[:], mybir.ActivationFunctionType.Copy
                )
                # SP ring: the ACT ring carries the W/id const loads.
                nc.sync.dma_start(
                    Od[b].rearrange("(t p) o -> p t o", p=128),
                    o_sb[:].rearrange("p (t o) -> p t o", t=NT),
                )

            def emit_pipeline():
                # software pipeline: prep(i+1), cs(i+1) and the A-load of i+1
                # are emitted before main(i); ff(i+1) right after main(i).
                v2_0 = emit_prep(0)
                cs_0 = emit_cs(0, v2_0)
                ff_cur = emit_ff(0, cs_0)
                a_cur = emit_a_load(0)
                for i in range(L):
                    if i + 1 < L:
                        a_next = emit_a_load(i + 1)  # first: SWDGE queue order
                        v2_next = emit_prep(i + 1)
                        cs_next = emit_cs(i + 1, v2_next)
                    emit_main(i, ff_cur, a_cur)
                    if i + 1 < L:
                        ff_cur = emit_ff(i + 1, cs_next)
                        a_cur = a_next

            if not nc_done:
                if loop:
                    with tc.For_i(0, reps, 1):
                        emit_pipeline()
                else:
                    emit_pipeline()

    nc.finalize()
    _NC_CACHE[key] = nc
    return nc


def make_const_inputs(fouriercoeffs, bias):
    import ml_dtypes

    W = np.ascontiguousarray(
        np.asarray(fouriercoeffs, np.float32)
        .transpose(0, 3, 2, 1)
        .reshape(2 * G * IN, OUT)
    ).astype(ml_dtypes.bfloat16)
    p = np.arange(128)
    gp = np.arange(G // 2)
    # k_g = g+1, g = 2*gp + p//64
    kscale = ((2 * gp[None, :] + p[:, None] // IN + 1) / (2.0 * np.pi)).astype(
        np.float32
    )
    ident = np.eye(128, dtype=np.float32)
    bias2 = np.tile(np.asarray(bias, np.float32).reshape(1, OUT), (1, 4)).astype(
        ml_dtypes.bfloat16
    )
    return W, kscale, ident, bias2


def kernel(V, A, fouriercoeffs, bias):
    nc = build_nc()
    W, kscale, ident, bias2 = make_const_inputs(fouriercoeffs, bias)
    V = np.asarray(V, np.float32)
    A = np.asarray(A, np.float32)
    in_maps = []
    for core in range(N_CORES):
        sl = slice(core * B_LOC, (core + 1) * B_LOC)
        in_maps.append(
            {
                "V": np.ascontiguousarray(V[sl]),
                "A": np.ascontiguousarray(A[sl]),
                "W": W,
                "kscale": kscale,
                "ident": ident,
                "bias2": bias2,
            }
        )
    res = run_bass_kernel_spmd(nc, in_maps, list(range(N_CORES)))
    return np.concatenate(
        [res.results[i]["out"] for i in range(N_CORES)], axis=0
    ).astype(np.float32)
